# revision 3
# baseline (speedup 1.0000x reference)
"""Trainium2 Bass kernel for nn_CGM (context-gated modulation).

Math (per batch element b):
    att[c,k]  = sum_hw feature[c,hw] * map[k,hw]          # [C,K] contraction
    scale[c]  = 1 + sum_k sigmoid(att[c,k]) * gamma[k]
    out[c,hw] = feature[c,hw] * scale[c]

Sharding: pure data parallel - one batch element per NeuronCore (B=8).

Strategy (v2; ~2.4x over the v1 PE-transpose kernel in same-conditions
A/B):
  - feature is staged on the HOST transposed and cast to fp16:
    ft[p, j*C + c] = feature[c, j*128 + p].  The hw-dim lands on SBUF
    partitions directly, so the att contraction needs NO PE transposes
    (v1 spent a full PE pass + a DVE/ACT copy pass on them), and HBM
    read traffic halves vs f32.  fp16 over bf16: same rates everywhere,
    ~8x finer mantissa (rel err 5e-4 vs 4e-3); the data range here
    (randn features, maps in [0,1], |att| < ~300 in f32 PSUM) is far
    inside fp16 limits.
  - att^T[k,c] accumulates in PSUM f32 over 128 hw-blocks directly:
    matmul(attT, mapt[:, j*KP:(j+1)*KP], ft[:, j*C:(j+1)*C]) in fp16,
    mapt being the host-transposed map with a zero pad column (KP=20).
  - scale broadcast tile bc[p,c] = 1 + sum_k gamma_k sigmoid(att[k,c])
    via one PE matmul with a gamma-broadcast stationary (its ones row
    folds the +1), then replicated along the free dim (rep).
  - out^T = ft * rep elementwise on DVE (16-bit 2x mode), stored fp16
    in the same transposed layout; the host un-transposes + casts f32.
  - HBM traffic per core: 9.1 MB in + 8.4 MB out (vs ~35 MB f32 in v1)
    -> DMA-roofline-bound; PE ~15us and DVE ~19us hide underneath.
  - Loads ride the SP HWDGE ring, stores the ACT ring, so waiting
    stores never head-of-line-block the next iteration's loads.
"""

import numpy as np
from contextlib import ExitStack
from types import SimpleNamespace

import concourse.bacc as bacc
import concourse.tile as tile
import concourse.mybir as mybir

B, C, K = 8, 256, 19
KP = 20               # K padded (pad col is zero in mapt; X row 19 = ones)
H = W = 128
HW = H * W            # 16384
P = 128               # SBUF partitions
NB = HW // P          # 128 hw blocks
NBC = NB * C          # 32768 transposed row length

F32 = mybir.dt.float32

KNOBS = dict(
    ch=4096,          # ft chunk width (elems): 4096 -> 1 MiB 16-bit chunks
    rings=2,          # 1: all DMA on SP; 2: loads on SP, stores on ACT
    tt_split=1,       # split each chunk's output multiply into N DVE ops
    rep_mode="mat",   # mat: materialize rep [P,CH]; bcast: stride-0 TT in1
    dtype="fp16",     # bf16 | fp16 (fp16: same rates, ~8x finer mantissa)
    mode="full",      # full | dma (loads+stores only) | att (loads+matmul)
)


def _dt():
    return {
        "bf16": mybir.dt.bfloat16,
        "fp16": mybir.dt.float16,
    }[KNOBS["dtype"]]


def _np_dt():
    import ml_dtypes

    return {"bf16": ml_dtypes.bfloat16, "fp16": np.float16}[KNOBS["dtype"]]

_prog_cache = {}
_runner_cache = {}


def _knobs_key(n_iters):
    return (n_iters,) + tuple(sorted(KNOBS.items()))


def _emit_body(nc, pools, ft_d, mapt_d, gb_d, out_d):
    sb, ftp, ps = pools
    DT16 = _dt()
    CH = KNOBS["ch"]
    NCH = NBC // CH
    BPC = CH // C          # hw-blocks per chunk
    MODE = KNOBS["mode"]
    DMA_ONLY = MODE == "dma"

    def qload():
        return nc.sync

    def qstore():
        return nc.scalar if KNOBS["rings"] == 2 else nc.sync

    if not DMA_ONLY:
        mapt = sb.tile([P, NB * KP], DT16, name="mapt", tag="mapt")
        nc.sync.dma_start(mapt[:], mapt_d[:])
        gb = sb.tile([KP, P], DT16, name="gb", tag="gb")
        nc.sync.dma_start(gb[:], gb_d[:])
        attT = ps.tile([KP, C], F32, name="attT", tag="attT")

    fts = []
    for j in range(NCH):
        t = ftp.tile([P, CH], DT16, name=f"ft{j}", tag="ft")
        qload().dma_start(t[:], ft_d[:, j * CH : (j + 1) * CH])
        fts.append(t)
        if not DMA_ONLY:
            for bb in range(BPC):
                blk = j * BPC + bb
                nc.tensor.matmul(
                    attT[:],
                    mapt[:, blk * KP : (blk + 1) * KP],
                    t[:, bb * C : (bb + 1) * C],
                    start=(blk == 0),
                    stop=(blk == NB - 1),
                )

    if MODE == "att":
        # attribution mode: force a tiny dependency on attT, store raw.
        X = sb.tile([KP, C], DT16, name="X", tag="X")
        nc.scalar.activation(
            X[0:K, :], attT[0:K, :], mybir.ActivationFunctionType.Sigmoid
        )
        for j in range(NCH):
            qstore().dma_start(out_d[:, j * CH : (j + 1) * CH], fts[j][:])
        return

    if not DMA_ONLY:
        # X[k,c] = sigmoid(att[k,c]) for k<19; row 19 stays ones.
        X = sb.tile([KP, C], DT16, name="X", tag="X")
        nc.vector.memset(X[:], 1.0)
        nc.scalar.activation(
            X[0:K, :], attT[0:K, :], mybir.ActivationFunctionType.Sigmoid
        )
        # bc[p,c] = sum_k gb[k,p] * X[k,c] = 1 + sum_k gamma_k sig(att[k,c])
        bcp = ps.tile([P, C], F32, name="bcp", tag="bcp")
        nc.tensor.matmul(bcp[:], gb[:], X[:], start=True, stop=True)
        if KNOBS["rep_mode"] == "mat":
            rep = sb.tile([P, CH], DT16, name="rep", tag="rep")
            nc.vector.tensor_copy(rep[:, 0:C], bcp[:])
            w = C
            while w < CH:
                nc.vector.tensor_copy(rep[:, w : 2 * w], rep[:, 0:w])
                w *= 2
        else:
            bc = sb.tile([P, C], DT16, name="bc", tag="bc")
            nc.vector.tensor_copy(bc[:], bcp[:])

    TS = KNOBS["tt_split"]
    TW = CH // TS
    for j in range(NCH):
        t = fts[j]
        for s in range(TS):
            cs = slice(s * TW, (s + 1) * TW)
            if not DMA_ONLY:
                if KNOBS["rep_mode"] == "mat":
                    nc.vector.tensor_mul(t[:, cs], t[:, cs], rep[:, cs])
                else:
                    src = (
                        bc[:]
                        .unsqueeze(1)
                        .broadcast_to([P, TW // C, C])
                    )
                    nc.vector.tensor_mul(
                        t[:, cs].rearrange(
                            "p (r c) -> p r c", r=TW // C
                        ),
                        t[:, cs].rearrange(
                            "p (r c) -> p r c", r=TW // C
                        ),
                        src,
                    )
            qstore().dma_start(
                out_d[:, j * CH + s * TW : j * CH + (s + 1) * TW], t[:, cs]
            )


def _build_program(n_iters=1):
    nc = bacc.Bacc("TRN2", target_bir_lowering=False, debug=False)

    DT16 = _dt()
    ft_d = nc.dram_tensor("ft", [P, NBC], DT16, kind="ExternalInput")
    mapt_d = nc.dram_tensor("mapt", [P, NB * KP], DT16, kind="ExternalInput")
    gb_d = nc.dram_tensor("gb", [KP, P], DT16, kind="ExternalInput")
    out_d = nc.dram_tensor("out", [P, NBC], DT16, kind="ExternalOutput")

    NCH = NBC // KNOBS["ch"]
    with tile.TileContext(nc) as tc, ExitStack() as ctx:
        pools = (
            ctx.enter_context(tc.tile_pool(name="sb", bufs=2)),
            ctx.enter_context(tc.tile_pool(name="ftp", bufs=2 * NCH)),
            ctx.enter_context(tc.tile_pool(name="ps", bufs=2, space="PSUM")),
        )
        for _ in range(n_iters):
            _emit_body(nc, pools, ft_d, mapt_d, gb_d, out_d)

    nc.compile()
    return nc


def get_program(n_iters=1):
    key = _knobs_key(n_iters)
    if key not in _prog_cache:
        _prog_cache[key] = _build_program(n_iters)
    return _prog_cache[key]


def make_runner(nc, n_cores=B):
    """Persistent jitted SPMD executor (no donation, staged device bufs)."""
    import jax
    from concourse import bass2jax
    from jax.experimental.shard_map import shard_map
    from jax.sharding import Mesh, NamedSharding, PartitionSpec

    bass2jax.install_neuronx_cc_hook()
    partition_name = (
        nc.partition_id_tensor.name if nc.partition_id_tensor else None
    )
    in_names, out_names, out_avals, zero_outs = [], [], [], []
    for alloc in nc.m.functions[0].allocations:
        if not isinstance(alloc, mybir.MemoryLocationSet):
            continue
        name = alloc.memorylocations[0].name
        if alloc.kind == "ExternalInput":
            if name != partition_name:
                in_names.append(name)
        elif alloc.kind == "ExternalOutput":
            out_names.append(name)
            shape = tuple(alloc.tensor_shape)
            dtype = mybir.dt.np(alloc.dtype)
            out_avals.append(jax.core.ShapedArray(shape, dtype))
            zero_outs.append(np.zeros(shape, dtype))
    n_params = len(in_names)
    all_in_names = list(in_names) + list(out_names)
    if partition_name is not None:
        all_in_names.append(partition_name)

    def _body(*args):
        operands = list(args)
        if partition_name is not None:
            operands.append(bass2jax.partition_id_tensor())
        outs = bass2jax._bass_exec_p.bind(
            *operands,
            out_avals=tuple(out_avals),
            in_names=tuple(all_in_names),
            out_names=tuple(out_names),
            lowering_input_output_aliases=(),
            sim_require_finite=True,
            sim_require_nnan=True,
            nc=nc,
        )
        return tuple(outs)

    devices = jax.devices()[:n_cores]
    mesh = Mesh(np.asarray(devices), ("core",))
    nsh = NamedSharding(mesh, PartitionSpec("core"))
    n_outs = len(out_names)
    sharded = jax.jit(
        shard_map(
            _body,
            mesh=mesh,
            in_specs=(PartitionSpec("core"),) * (n_params + n_outs),
            out_specs=(PartitionSpec("core"),) * n_outs,
            check_rep=False,
        ),
        keep_unused=True,
    )

    def stage(in_maps):
        assert len(in_maps) == n_cores
        arrs = [
            np.concatenate([np.asarray(m[n]) for m in in_maps], axis=0)
            for n in in_names
        ]
        arrs += [
            np.zeros((n_cores * z.shape[0], *z.shape[1:]), z.dtype)
            for z in zero_outs
        ]
        return [jax.device_put(a, nsh) for a in arrs]

    def call(staged):
        outs = sharded(*staged)
        jax.block_until_ready(outs)
        return outs

    def unpack(outs):
        res = []
        for c in range(n_cores):
            res.append(
                {
                    name: np.asarray(outs[i]).reshape(
                        n_cores, *out_avals[i].shape
                    )[c]
                    for i, name in enumerate(out_names)
                }
            )
        return res

    return SimpleNamespace(
        stage=stage, call=call, unpack=unpack, sharded=sharded
    )


def get_runner(n_iters=1):
    key = _knobs_key(n_iters)
    if key not in _runner_cache:
        _runner_cache[key] = make_runner(get_program(n_iters))
    return _runner_cache[key]


def make_in_maps(feature, map, gamma):
    """Host-side sharding + layout prep.

    feature [B,C,H,W] f32, map [B,K,H,W] f32, gamma [1,1,1,1,K] f32.
    Per core b:
      ft[p, j*C + c]   = bf16(feature[b, c, j*128 + p])
      mapt[p, j*KP + k] = bf16(map[b, k, j*128 + p]), zero pad k=19
      gb[k, p] = gamma[k] (k<19) | 1.0 (k=19)
    """
    bf16 = _np_dt()
    feature = np.asarray(feature, dtype=np.float32)
    map = np.asarray(map, dtype=np.float32)
    gamma = np.asarray(gamma, dtype=np.float32)

    gb = np.ones((KP, P), np.float32)
    gb[:K, :] = gamma.reshape(K, 1)
    gb = gb.astype(bf16)

    in_maps = []
    for b in range(B):
        ft = (
            feature[b]
            .reshape(C, NB, P)
            .transpose(2, 1, 0)
            .astype(bf16)
            .reshape(P, NBC)
        )
        m_b = np.zeros((P, NB, KP), np.float32)
        m_b[:, :, :K] = map[b].reshape(K, NB, P).transpose(2, 1, 0)
        in_maps.append(
            {
                "ft": np.ascontiguousarray(ft),
                "mapt": np.ascontiguousarray(
                    m_b.astype(bf16).reshape(P, NB * KP)
                ),
                "gb": gb,
            }
        )
    return in_maps


def run(inputs, n_iters=1):
    runner = get_runner(n_iters)
    in_maps = make_in_maps(inputs["feature"], inputs["map"], inputs["gamma"])
    staged = runner.stage(in_maps)
    outs = runner.call(staged)
    res = runner.unpack(outs)
    out = np.empty((B, C, H, W), dtype=np.float32)
    for b in range(B):
        # out_t[p, j*C + c] -> out[c, j*128 + p]
        out[b] = (
            res[b]["out"]
            .reshape(P, NB, C)
            .transpose(2, 1, 0)
            .astype(np.float32)
            .reshape(C, H, W)
        )
    return out


def kernel(**inputs):
    return run(inputs)


if __name__ == "__main__":
    rng = np.random.default_rng(0)
    inputs = {
        "feature": rng.standard_normal((B, C, H, W), dtype=np.float32),
        "map": rng.random((B, K, H, W), dtype=np.float32),
        "gamma": (rng.standard_normal((1, 1, 1, 1, K)) * 0.1).astype(
            np.float32
        ),
    }
    out = kernel(**inputs)
    print("out", out.shape, out.dtype)
